# revision 13
# baseline (speedup 1.0000x reference)
"""Trainium2 Bass kernel for nn_EternalNeuralNetwork.

Network (B=4096, IN=1024, H=2048, OUT=1024, DEPTH=9):
    h = relu(x @ w0 + b0)
    h = relu(h @ w1 + b1)
    h = tanh(h @ e1_cw + e1_cb + eternal1)      # eternal layer 1
    h = relu(h @ w2 + b2)
    h = tanh(h @ e2_cw + e2_cb + eternal2)      # eternal layer 2
    out = relu(h @ w3 + b3)

The "eternal" branch collapses analytically: the per-j state starts as a
constant vector (1/sqrt(d) everywhere) and each rotation gate maps a constant
vector s to cos(angle)*s (the two roll terms cancel), so
    eternal[j] = (prod_{t,k} cos(ew[t, j, k]))^2 / d
i.e. a per-output-feature scalar computed from the 27 angles of feature j.
It is computed on-device from the [2048, 27] angle slice via the ACT engine
(Sin LUT with +pi/2 bias) and a log2 tree of DVE multiplies.

Sharding: pure data-parallel over batch, 512 rows per core, weights
replicated. All activations are kept TRANSPOSED on device ([features, batch]
with features on partitions) so every GEMM uses the weight matrix directly as
the stationary operand and the activations as the moving operand — no
on-device transposes anywhere. Matmuls run as float32r (full fp32 data,
single-pass reduced-precision multiply) which streams at 1 row/cycle for
moving dim >= 256, i.e. bf16-rate on fp32 data.
"""

import math

import numpy as np

import concourse.bass as bass  # noqa: F401  (bass types used via tile/bacc)
import concourse.tile as tile
from concourse import bacc, mybir
from concourse.bass_utils import run_bass_kernel_spmd

P = 128
N_CORES = 8
B, IN, H, OUT = 4096, 1024, 2048, 1024
NB = B // N_CORES  # 512 batch columns per core
DEPTH = 3 * 9  # 27 angles per output feature
ANG_PAD = 32  # padded to a power of two for the product tree
F32 = mybir.dt.float32
F32R = mybir.dt.float32r
MCHUNK = 4  # m-tiles (128 output features each) per weight chunk

# (weight key, bias key, K, M, activation, eternal angles key or None)
LAYERS = [
    ("w0", "b0", IN, H, "relu", None),
    ("w1", "b1", H, H, "relu", None),
    ("e1_cw", "e1_cb", H, H, "tanh", "e1_ang"),
    ("w2", "b2", H, H, "relu", None),
    ("e2_cw", "e2_cb", H, H, "tanh", "e2_ang"),
    ("w3", "b3", H, OUT, "relu", None),
]

_CACHE = {}


def _pack_w(w: np.ndarray) -> np.ndarray:
    """[K, M] -> [n_chunks*128, kt_n*MCHUNK*128] with per-partition-contiguous
    chunk layout: packed[c*128+p, kt*512+m] = w[kt*128+p, c*512+m]."""
    K, M = w.shape
    kt_n = K // P
    ckn = M // (MCHUNK * P)
    w4 = w.reshape(kt_n, P, ckn, MCHUNK * P)
    return np.ascontiguousarray(
        w4.transpose(2, 1, 0, 3).reshape(ckn * P, kt_n * MCHUNK * P)
    )


def _pack_rows(v: np.ndarray, cols: int) -> np.ndarray:
    """[F, cols] -> [128, (F/128)*cols]: packed[p, ft*cols+c] = v[ft*128+p, c]."""
    F = v.shape[0]
    return np.ascontiguousarray(
        v.reshape(F // P, P, cols).transpose(1, 0, 2).reshape(P, (F // P) * cols)
    )


def _pack_bias(b: np.ndarray) -> np.ndarray:
    return _pack_rows(b.reshape(-1, 1), 1)


def _pack_angles(ew: np.ndarray) -> np.ndarray:
    """ew [9, 2048, 2048] -> padded [128, 16*32] angle tile.

    Only ew[:, j, 0:3] is used by the reference. Column 27 is set to
    arccos(1/sqrt(2048)) so that (prod of 32 cosines)^2 == (prod of 27)^2/2048;
    columns 28-31 are 0 (cos(0) == 1)."""
    ang = ew[:, :, :3].transpose(1, 0, 2).reshape(H, DEPTH).astype(np.float32)
    pad = np.zeros((H, ANG_PAD), np.float32)
    pad[:, :DEPTH] = ang
    pad[:, DEPTH] = math.acos(1.0 / math.sqrt(H))
    return _pack_rows(pad, ANG_PAD)


def _build():
    nc = bacc.Bacc("TRN2", target_bir_lowering=False, debug=False)

    dram = {}
    for wk, bk, K, M, _act, angk in LAYERS:
        ckn = M // (MCHUNK * P)
        dram[wk] = nc.dram_tensor(wk, [ckn * P, (K // P) * MCHUNK * P], F32,
                                  kind="ExternalInput")
        dram[bk] = nc.dram_tensor(bk, [P, M // P], F32, kind="ExternalInput")
        if angk is not None:
            dram[angk] = nc.dram_tensor(angk, [P, (H // P) * ANG_PAD], F32,
                                        kind="ExternalInput")
    dram["xT"] = nc.dram_tensor("xT", [P, (IN // P) * NB], F32, kind="ExternalInput")
    dram["outT"] = nc.dram_tensor("outT", [P, (OUT // P) * NB], F32,
                                  kind="ExternalOutput")

    with tile.TileContext(nc) as tc:
        with (
            tc.tile_pool(name="acts", bufs=2) as acts,
            tc.tile_pool(name="wpool", bufs=3) as wpool,
            tc.tile_pool(name="small", bufs=1) as small,
            tc.tile_pool(name="tmp", bufs=2) as tmp,
            tc.tile_pool(name="psum", bufs=7, space="PSUM") as psum,
        ):
            # ---- PE warm-up: dummy matmuls on memset tiles so the HAM clock
            # gate releases (1.2 -> 2.4 GHz) while the first DMAs stream in.
            wu_w = small.tile([P, P], F32, tag="wu_w")
            nc.vector.memset(wu_w, 0.0)
            wu_a = small.tile([P, NB], F32, tag="wu_a")
            nc.vector.memset(wu_a, 0.0)
            wu_ps = psum.tile([P, NB], F32, tag="wu_ps", bufs=1)
            for _ in range(4):
                # plain fp32 matmuls (4 cyc/row): ~1.7us each cold, enough to
                # release the HAM clock gate and keep the PE busy until the
                # first real matmul's data has landed (no idle re-throttle)
                nc.tensor.matmul(wu_ps, lhsT=wu_w, rhs=wu_a, start=True, stop=True)

            # ---- input activations (transposed, packed); split DMA so the
            # first k-tiles land (and matmuls can start) early.
            act_in = acts.tile([P, (IN // P) * NB], F32R, tag="act")
            xv = dram["xT"].ap().bitcast(F32R)
            xq = (IN // P) * NB // 4
            for qi in range(4):
                nc.sync.dma_start(
                    out=act_in[:, qi * xq : (qi + 1) * xq],
                    in_=xv[:, qi * xq : (qi + 1) * xq],
                )

            bias_sb = {}
            b0 = small.tile([P, H // P], F32, tag="b_b0")
            nc.sync.dma_start(out=b0, in_=dram["b0"].ap())
            bias_sb["b0"] = b0

            comb = {}
            halfpi = small.tile([P, 1], F32, tag="halfpi")
            nc.vector.memset(halfpi, math.pi / 2.0)

            def emit_eternal_biases():
                """cb + (prod cos)^2/H per output feature, both eternal layers.
                Emitted after layer 0 so its DMAs don't delay xT/w0."""
                for _wk, bk, _K, _M, _act, angk in LAYERS:
                    if angk is None:
                        continue
                    ft_n = H // P
                    angt = small.tile([P, ft_n * ANG_PAD], F32, tag=f"ang_{angk}")
                    nc.sync.dma_start(out=angt, in_=dram[angk].ap())
                    cbt = small.tile([P, ft_n], F32, tag=f"cb_{bk}")
                    nc.sync.dma_start(out=cbt, in_=dram[bk].ap())
                    combt = small.tile([P, ft_n], F32, tag=f"comb_{angk}")
                    for ft in range(ft_n):
                        cos = tmp.tile([P, ANG_PAD], F32, tag="cos")
                        nc.scalar.activation(
                            out=cos,
                            in_=angt[:, ft * ANG_PAD : (ft + 1) * ANG_PAD],
                            func=mybir.ActivationFunctionType.Sin,
                            bias=halfpi,
                            scale=1.0,
                        )
                        # product tree: 32 -> 16 -> 8 -> 4 -> 2 -> 1
                        w = ANG_PAD
                        cur = cos
                        while w > 1:
                            w //= 2
                            nxt = tmp.tile([P, w], F32, tag=f"pt{w}")
                            nc.vector.tensor_mul(nxt, cur[:, :w], cur[:, w : 2 * w])
                            cur = nxt
                        sq = tmp.tile([P, 1], F32, tag="sq")
                        nc.vector.tensor_mul(sq, cur, cur)
                        nc.vector.tensor_add(
                            combt[:, ft : ft + 1], sq, cbt[:, ft : ft + 1]
                        )
                    comb[angk] = combt
                for _wk, bk, _K, M, _act, angk in LAYERS[1:]:
                    if angk is None:
                        bt = small.tile([P, M // P], F32, tag=f"b_{bk}")
                        nc.sync.dma_start(out=bt, in_=dram[bk].ap())
                        bias_sb[bk] = bt

            # ---- the 6 GEMM layers
            for li, (wk, bk, K, M, act, angk) in enumerate(LAYERS):
                kt_n = K // P
                ckn = M // (MCHUNK * P)
                last = li == len(LAYERS) - 1
                act_out = acts.tile(
                    [P, (M // P) * NB], F32 if last else F32R, tag="act"
                )
                func = (
                    mybir.ActivationFunctionType.Tanh
                    if act == "tanh"
                    else mybir.ActivationFunctionType.Relu
                )
                bt = comb[angk] if angk is not None else bias_sb[bk]
                wv = dram[wk].ap().bitcast(F32R)
                first_epilogue = None
                for ci in range(ckn):
                    wt = wpool.tile([P, kt_n * MCHUNK * P], F32R, tag="w")
                    # split so the first k-tiles (leading free-dim) land early;
                    # layer 0 is on the critical path so split it finer
                    nsplit = 4 if li == 0 else 2
                    wq = kt_n * MCHUNK * P // nsplit
                    for qi in range(nsplit):
                        d = nc.sync.dma_start(
                            out=wt[:, qi * wq : (qi + 1) * wq],
                            in_=wv[ci * P : (ci + 1) * P, qi * wq : (qi + 1) * wq],
                        )
                        if li == 0 and ci > 0 and first_epilogue is not None:
                            # keep layer-0 prefetches off the critical first
                            # 2MB (xT + chunk 0) — they'd steal HBM bandwidth
                            tile.add_dep_helper(
                                d.ins,
                                first_epilogue.ins,
                                sync=True,
                                reason="defer L0 prefetch behind first epilogue",
                            )
                    for mj in range(MCHUNK):
                        mt = ci * MCHUNK + mj
                        ps = psum.tile([P, NB], F32, tag="ps")
                        for kt in range(kt_n):
                            nc.tensor.matmul(
                                ps,
                                lhsT=wt[
                                    :,
                                    kt * MCHUNK * P
                                    + mj * P : kt * MCHUNK * P
                                    + (mj + 1) * P,
                                ],
                                rhs=act_in[:, kt * NB : (kt + 1) * NB],
                                start=(kt == 0),
                                stop=(kt == kt_n - 1),
                            )
                        ep = nc.scalar.activation(
                            out=act_out[:, mt * NB : (mt + 1) * NB],
                            in_=ps,
                            func=func,
                            bias=bt[:, mt : mt + 1],
                            scale=1.0,
                        )
                        if first_epilogue is None:
                            first_epilogue = ep
                        if last:
                            # stream each output m-tile as soon as it's done
                            nc.sync.dma_start(
                                out=dram["outT"].ap()[:, mt * NB : (mt + 1) * NB],
                                in_=act_out[:, mt * NB : (mt + 1) * NB],
                            )
                act_in = act_out
                if li == 0:
                    emit_eternal_biases()

    nc.compile()
    return nc


def _prepare_in_maps(inputs):
    x = np.asarray(inputs["x"], np.float32)
    shared = {}
    for wk, bk, _K, _M, _act, angk in LAYERS:
        shared[wk] = _pack_w(np.asarray(inputs[wk], np.float32))
        shared[bk] = _pack_bias(np.asarray(inputs[bk], np.float32))
        if angk is not None:
            ewk = "e1_ew" if angk == "e1_ang" else "e2_ew"
            shared[angk] = _pack_angles(np.asarray(inputs[ewk], np.float32))
    in_maps = []
    for c in range(N_CORES):
        xs = x[c * NB : (c + 1) * NB, :]  # [512, 1024]
        xT = np.ascontiguousarray(
            xs.T.reshape(IN // P, P, NB).transpose(1, 0, 2).reshape(P, -1)
        )
        in_maps.append({**shared, "xT": xT})
    return in_maps


def kernel(trace=False, **inputs) -> np.ndarray:
    if "nc" not in _CACHE:
        _CACHE["nc"] = _build()
    nc = _CACHE["nc"]

    in_maps = _prepare_in_maps(inputs)
    res = run_bass_kernel_spmd(
        nc, in_maps, core_ids=list(range(N_CORES)), trace=trace
    )
    _CACHE["last_result"] = res

    out = np.empty((B, OUT), np.float32)
    for c in range(N_CORES):
        oT = res.results[c]["outT"]  # [128, 8*512] packed
        o = oT.reshape(P, OUT // P, NB).transpose(1, 0, 2).reshape(OUT, NB)
        out[c * NB : (c + 1) * NB, :] = o.T
    return out


# revision 15
# speedup vs baseline: 1.0248x; 1.0248x over previous
"""Trainium2 Bass kernel for nn_EternalNeuralNetwork.

Network (B=4096, IN=1024, H=2048, OUT=1024, DEPTH=9):
    h = relu(x @ w0 + b0)
    h = relu(h @ w1 + b1)
    h = tanh(h @ e1_cw + e1_cb + eternal1)      # eternal layer 1
    h = relu(h @ w2 + b2)
    h = tanh(h @ e2_cw + e2_cb + eternal2)      # eternal layer 2
    out = relu(h @ w3 + b3)

The "eternal" branch collapses analytically: the per-j state starts as a
constant vector (1/sqrt(d) everywhere) and each rotation gate maps a constant
vector s to cos(angle)*s (the two roll terms cancel), so
    eternal[j] = (prod_{t,k} cos(ew[t, j, k]))^2 / d
i.e. a per-output-feature scalar computed from the 27 angles of feature j.
It is computed on-device from the [2048, 27] angle slice via the ACT engine
(Sin LUT with +pi/2 bias) and a log2 tree of DVE multiplies.

Sharding: pure data-parallel over batch, 512 rows per core, weights
replicated. All activations are kept TRANSPOSED on device ([features, batch]
with features on partitions) so every GEMM uses the weight matrix directly as
the stationary operand and the activations as the moving operand — no
on-device transposes anywhere. Matmuls run as float32r (full fp32 data,
single-pass reduced-precision multiply) which streams at 1 row/cycle for
moving dim >= 256, i.e. bf16-rate on fp32 data.
"""

import math

import numpy as np

import concourse.bass as bass  # noqa: F401  (bass types used via tile/bacc)
import concourse.tile as tile
from concourse import bacc, mybir
from concourse.bass_utils import run_bass_kernel_spmd

P = 128
N_CORES = 8
B, IN, H, OUT = 4096, 1024, 2048, 1024
NB = B // N_CORES  # 512 batch columns per core
DEPTH = 3 * 9  # 27 angles per output feature
ANG_PAD = 32  # padded to a power of two for the product tree
F32 = mybir.dt.float32
F32R = mybir.dt.float32r
MCHUNK = 4  # m-tiles (128 output features each) per weight chunk

# (weight key, bias key, K, M, activation, eternal angles key or None)
LAYERS = [
    ("w0", "b0", IN, H, "relu", None),
    ("w1", "b1", H, H, "relu", None),
    ("e1_cw", "e1_cb", H, H, "tanh", "e1_ang"),
    ("w2", "b2", H, H, "relu", None),
    ("e2_cw", "e2_cb", H, H, "tanh", "e2_ang"),
    ("w3", "b3", H, OUT, "relu", None),
]

_CACHE = {}


def _pack_w(w: np.ndarray) -> np.ndarray:
    """[K, M] -> [n_chunks*128, kt_n*MCHUNK*128] with per-partition-contiguous
    chunk layout: packed[c*128+p, kt*512+m] = w[kt*128+p, c*512+m]."""
    K, M = w.shape
    kt_n = K // P
    ckn = M // (MCHUNK * P)
    w4 = w.reshape(kt_n, P, ckn, MCHUNK * P)
    return np.ascontiguousarray(
        w4.transpose(2, 1, 0, 3).reshape(ckn * P, kt_n * MCHUNK * P)
    )


def _pack_rows(v: np.ndarray, cols: int) -> np.ndarray:
    """[F, cols] -> [128, (F/128)*cols]: packed[p, ft*cols+c] = v[ft*128+p, c]."""
    F = v.shape[0]
    return np.ascontiguousarray(
        v.reshape(F // P, P, cols).transpose(1, 0, 2).reshape(P, (F // P) * cols)
    )


def _pack_bias(b: np.ndarray) -> np.ndarray:
    return _pack_rows(b.reshape(-1, 1), 1)


def _pack_angles(ew: np.ndarray) -> np.ndarray:
    """ew [9, 2048, 2048] -> padded [128, 16*32] angle tile.

    Only ew[:, j, 0:3] is used by the reference. Column 27 is set to
    arccos(1/sqrt(2048)) so that (prod of 32 cosines)^2 == (prod of 27)^2/2048;
    columns 28-31 are 0 (cos(0) == 1)."""
    ang = ew[:, :, :3].transpose(1, 0, 2).reshape(H, DEPTH).astype(np.float32)
    pad = np.zeros((H, ANG_PAD), np.float32)
    pad[:, :DEPTH] = ang
    pad[:, DEPTH] = math.acos(1.0 / math.sqrt(H))
    return _pack_rows(pad, ANG_PAD)


def _build():
    nc = bacc.Bacc("TRN2", target_bir_lowering=False, debug=False)

    dram = {}
    for wk, bk, K, M, _act, angk in LAYERS:
        ckn = M // (MCHUNK * P)
        dram[wk] = nc.dram_tensor(wk, [ckn * P, (K // P) * MCHUNK * P], F32,
                                  kind="ExternalInput")
        dram[bk] = nc.dram_tensor(bk, [P, M // P], F32, kind="ExternalInput")
        if angk is not None:
            dram[angk] = nc.dram_tensor(angk, [P, (H // P) * ANG_PAD], F32,
                                        kind="ExternalInput")
    dram["xT"] = nc.dram_tensor("xT", [P, (IN // P) * NB], F32, kind="ExternalInput")
    dram["outT"] = nc.dram_tensor("outT", [P, (OUT // P) * NB], F32,
                                  kind="ExternalOutput")

    with tile.TileContext(nc) as tc:
        with (
            tc.tile_pool(name="acts", bufs=2) as acts,
            tc.tile_pool(name="wpool", bufs=3) as wpool,
            tc.tile_pool(name="small", bufs=1) as small,
            tc.tile_pool(name="tmp", bufs=2) as tmp,
            tc.tile_pool(name="psum", bufs=7, space="PSUM") as psum,
        ):
            # ---- PE warm-up: dummy matmuls on memset tiles so the HAM clock
            # gate releases (1.2 -> 2.4 GHz) while the first DMAs stream in.
            wu_w = small.tile([P, P], F32, tag="wu_w")
            nc.vector.memset(wu_w, 0.0)
            wu_a = small.tile([P, NB], F32, tag="wu_a")
            nc.vector.memset(wu_a, 0.0)
            wu_ps = psum.tile([P, NB], F32, tag="wu_ps", bufs=1)
            for _ in range(6):
                # plain fp32 matmuls (4 cyc/row): ~1.7us each cold, enough to
                # release the HAM clock gate and keep the PE busy until the
                # first real matmul's data has landed (no idle re-throttle)
                nc.tensor.matmul(wu_ps, lhsT=wu_w, rhs=wu_a, start=True, stop=True)

            # ---- input activations (transposed, packed); split DMA so the
            # first k-tiles land (and matmuls can start) early.
            act_in = acts.tile([P, (IN // P) * NB], F32R, tag="act")
            xv = dram["xT"].ap().bitcast(F32R)
            xq = (IN // P) * NB // 4
            for qi in range(4):
                nc.sync.dma_start(
                    out=act_in[:, qi * xq : (qi + 1) * xq],
                    in_=xv[:, qi * xq : (qi + 1) * xq],
                )

            bias_sb = {}
            b0 = small.tile([P, H // P], F32, tag="b_b0")
            nc.sync.dma_start(out=b0, in_=dram["b0"].ap())
            bias_sb["b0"] = b0

            comb = {}
            halfpi = small.tile([P, 1], F32, tag="halfpi")
            nc.vector.memset(halfpi, math.pi / 2.0)

            def emit_eternal_biases():
                """cb + (prod cos)^2/H per output feature, both eternal layers.
                Emitted after layer 0 so its DMAs don't delay xT/w0."""
                for _wk, bk, _K, _M, _act, angk in LAYERS:
                    if angk is None:
                        continue
                    ft_n = H // P
                    angt = small.tile([P, ft_n * ANG_PAD], F32, tag=f"ang_{angk}")
                    nc.sync.dma_start(out=angt, in_=dram[angk].ap())
                    cbt = small.tile([P, ft_n], F32, tag=f"cb_{bk}")
                    nc.sync.dma_start(out=cbt, in_=dram[bk].ap())
                    combt = small.tile([P, ft_n], F32, tag=f"comb_{angk}")
                    for ft in range(ft_n):
                        cos = tmp.tile([P, ANG_PAD], F32, tag="cos")
                        nc.scalar.activation(
                            out=cos,
                            in_=angt[:, ft * ANG_PAD : (ft + 1) * ANG_PAD],
                            func=mybir.ActivationFunctionType.Sin,
                            bias=halfpi,
                            scale=1.0,
                        )
                        # product tree: 32 -> 16 -> 8 -> 4 -> 2 -> 1
                        w = ANG_PAD
                        cur = cos
                        while w > 1:
                            w //= 2
                            nxt = tmp.tile([P, w], F32, tag=f"pt{w}")
                            nc.vector.tensor_mul(nxt, cur[:, :w], cur[:, w : 2 * w])
                            cur = nxt
                        sq = tmp.tile([P, 1], F32, tag="sq")
                        nc.vector.tensor_mul(sq, cur, cur)
                        nc.vector.tensor_add(
                            combt[:, ft : ft + 1], sq, cbt[:, ft : ft + 1]
                        )
                    comb[angk] = combt
                for _wk, bk, _K, M, _act, angk in LAYERS[1:]:
                    if angk is None:
                        bt = small.tile([P, M // P], F32, tag=f"b_{bk}")
                        nc.sync.dma_start(out=bt, in_=dram[bk].ap())
                        bias_sb[bk] = bt

            # ---- the 6 GEMM layers
            for li, (wk, bk, K, M, act, angk) in enumerate(LAYERS):
                kt_n = K // P
                ckn = M // (MCHUNK * P)
                last = li == len(LAYERS) - 1
                act_out = acts.tile(
                    [P, (M // P) * NB], F32 if last else F32R, tag="act"
                )
                func = (
                    mybir.ActivationFunctionType.Tanh
                    if act == "tanh"
                    else mybir.ActivationFunctionType.Relu
                )
                bt = comb[angk] if angk is not None else bias_sb[bk]
                wv = dram[wk].ap().bitcast(F32R)
                first_epilogue = None
                for ci in range(ckn):
                    wt = wpool.tile([P, kt_n * MCHUNK * P], F32R, tag="w")
                    # split so the first k-tiles (leading free-dim) land early;
                    # layer 0 is on the critical path so split it finer
                    nsplit = 4 if li == 0 else 2
                    wq = kt_n * MCHUNK * P // nsplit
                    for qi in range(nsplit):
                        nc.sync.dma_start(
                            out=wt[:, qi * wq : (qi + 1) * wq],
                            in_=wv[ci * P : (ci + 1) * P, qi * wq : (qi + 1) * wq],
                        )
                    for mj in range(MCHUNK):
                        mt = ci * MCHUNK + mj
                        ps = psum.tile([P, NB], F32, tag="ps")
                        for kt in range(kt_n):
                            nc.tensor.matmul(
                                ps,
                                lhsT=wt[
                                    :,
                                    kt * MCHUNK * P
                                    + mj * P : kt * MCHUNK * P
                                    + (mj + 1) * P,
                                ],
                                rhs=act_in[:, kt * NB : (kt + 1) * NB],
                                start=(kt == 0),
                                stop=(kt == kt_n - 1),
                            )
                        ep = nc.scalar.activation(
                            out=act_out[:, mt * NB : (mt + 1) * NB],
                            in_=ps,
                            func=func,
                            bias=bt[:, mt : mt + 1],
                            scale=1.0,
                        )
                        if first_epilogue is None:
                            first_epilogue = ep
                        if last:
                            # stream each output m-tile as soon as it's done
                            nc.sync.dma_start(
                                out=dram["outT"].ap()[:, mt * NB : (mt + 1) * NB],
                                in_=act_out[:, mt * NB : (mt + 1) * NB],
                            )
                act_in = act_out
                if li == 0:
                    emit_eternal_biases()

    nc.compile()
    return nc


def _prepare_in_maps(inputs):
    x = np.asarray(inputs["x"], np.float32)
    shared = {}
    for wk, bk, _K, _M, _act, angk in LAYERS:
        shared[wk] = _pack_w(np.asarray(inputs[wk], np.float32))
        shared[bk] = _pack_bias(np.asarray(inputs[bk], np.float32))
        if angk is not None:
            ewk = "e1_ew" if angk == "e1_ang" else "e2_ew"
            shared[angk] = _pack_angles(np.asarray(inputs[ewk], np.float32))
    in_maps = []
    for c in range(N_CORES):
        xs = x[c * NB : (c + 1) * NB, :]  # [512, 1024]
        xT = np.ascontiguousarray(
            xs.T.reshape(IN // P, P, NB).transpose(1, 0, 2).reshape(P, -1)
        )
        in_maps.append({**shared, "xT": xT})
    return in_maps


def kernel(trace=False, **inputs) -> np.ndarray:
    if "nc" not in _CACHE:
        _CACHE["nc"] = _build()
    nc = _CACHE["nc"]

    in_maps = _prepare_in_maps(inputs)
    res = run_bass_kernel_spmd(
        nc, in_maps, core_ids=list(range(N_CORES)), trace=trace
    )
    _CACHE["last_result"] = res

    out = np.empty((B, OUT), np.float32)
    for c in range(N_CORES):
        oT = res.results[c]["outT"]  # [128, 8*512] packed
        o = oT.reshape(P, OUT // P, NB).transpose(1, 0, 2).reshape(OUT, NB)
        out[c * NB : (c + 1) * NB, :] = o.T
    return out
